# revision 1
# baseline (speedup 1.0000x reference)
"""DiGCN link prediction on 8 TRN2 NeuronCores.

Math (reference):
    h1 = relu(segsum_dst(w_e * (x@W1)[src]) + b1)
    h2 = segsum_dst(w_e * (h1@W2)[src]) + b2
    logits = concat(h2[qs], h2[qd]) @ Wl + bl ; out = log_softmax(logits)

Device strategy (per core, SPMD-identical graph, per-core data):
  - dst-sharded edges. Host packs each core's dsts into fixed 16-column
    windows (whole dsts, FFD), 32 windows per 512-col PSUM group.
  - Layer 1 uses linearity: segsum(w, x@W1) = segsum(w, x)@W1. Per window
    two 128-slot gather blocks (src<25000 and src>=25000 halves, int16
    dma_gather from the two x table halves), per-block matmul
    msgs^T[128e,128f] @ S[128e,16] into feature-major PSUM; the hi pass
    adds on top in SBUF. Then project W1^T (f32r) + relu + b1 -> h1T.
  - Layer 2 + head use linearity again: with A=W2@Wl[:256], B=W2@Wl[256:],
    u[d]=sum w_e*(h1@A)[src]+b2@Wlt, v[d]=sum w_e*(h1@B)[src]+b2@Wlb+bl,
    logits[q] = u[qs]+v[qd]. yab=h1@[A|B] ([*,4] bf16) is AllGathered
    (1 MB), padded locally into a 256B-row table, aggregated with the same
    window structure (3 sub-blocks per window, split by src owner-core
    group for int16).
  - Query head: queries are processed where qs lives (local u gather from
    a padded 256B-row u table), the u-halves are AllToAll'd to the qd
    owner, which gathers v locally, adds, and takes log_softmax.
"""

import math
from contextlib import ExitStack

import ml_dtypes
import numpy as np

import concourse.bass as bass
import concourse.tile as tile
from concourse import bacc, mybir
from concourse.masks import make_identity

BF16 = mybir.dt.bfloat16
F32 = mybir.dt.float32
F32R = mybir.dt.float32r
I16 = mybir.dt.int16
P = 128
WCOLS = 16      # columns per window
GW = 32         # windows per PSUM group (512 cols)


def _wrap_idx(stream):
    """[n] int -> [128, n//16] int16 in dma_gather wrapped layout."""
    n = len(stream)
    a = np.asarray(stream, np.int64).reshape(n // 16, 16).T
    return np.tile(a, (8, 1)).astype(np.int16)


def plan(inputs, n_cores=8, verbose=False):
    x = np.asarray(inputs["x"], np.float32)
    edge_index = np.asarray(inputs["edge_index"], np.int64)
    query_edges = np.asarray(inputs["query_edges"], np.int64)
    edge_weight = np.asarray(inputs["edge_weight"], np.float32)
    W1 = np.asarray(inputs["W1"], np.float32)
    b1 = np.asarray(inputs["b1"], np.float32)
    W2 = np.asarray(inputs["W2"], np.float32)
    b2 = np.asarray(inputs["b2"], np.float32)
    Wl = np.asarray(inputs["Wl"], np.float32)
    bl = np.asarray(inputs["bl"], np.float32)

    N, F = x.shape
    E = edge_index.shape[1]
    Q = query_edges.shape[0]
    assert F == 256 and N % n_cores == 0
    n_local = N // n_cores
    NH = N // 2                      # x table split point
    cg = [0, 3 * n_local * 3 // 3, 0, 0]  # placeholder
    # owner-core groups for the L2 table third-split: {0,1,2},{3,4,5},{6,7}
    g_of_core = np.array([0, 0, 0, 1, 1, 1, 2, 2][:n_cores])
    tb = [np.searchsorted(g_of_core, t) * n_local for t in range(3)]
    tb.append(N)  # third t covers nodes [tb[t], tb[t+1])

    src = edge_index[0]
    dst = edge_index[1]
    qs, qd = query_edges[:, 0], query_edges[:, 1]

    # ---- pack windows per core (FFD by degree desc, 6 caps) ----
    CAP = 126
    packs = []
    W = 0
    for c in range(n_cores):
        m = dst // n_local == c
        ed = dst[m] - c * n_local
        es = src[m]
        deg = np.bincount(ed, minlength=n_local)
        lo1 = np.bincount(ed[es < NH], minlength=n_local)
        t0 = np.bincount(ed[es < tb[1]], minlength=n_local)
        t1 = np.bincount(ed[(es >= tb[1]) & (es < tb[2])], minlength=n_local)
        hi1 = deg - lo1
        t2 = deg - t0 - t1
        assert max(lo1.max(), hi1.max(), t0.max(), t1.max(), t2.max()) <= CAP
        order = np.argsort(-deg, kind="stable")
        caps = np.stack([lo1, hi1, t0, t1, t2], 1)  # [n_local, 5]
        win_of = np.empty(n_local, np.int64)
        rank_of = np.empty(n_local, np.int64)
        wins_used = []   # list of [5] counts
        wins_n = []      # dsts per window
        SCAN = 24
        for d in order:
            cd = caps[d]
            placed = False
            for wi in range(len(wins_used) - 1, max(-1, len(wins_used) - 1 - SCAN), -1):
                if wins_n[wi] < WCOLS and np.all(wins_used[wi] + cd <= CAP):
                    win_of[d] = wi
                    rank_of[d] = wins_n[wi]
                    wins_used[wi] += cd
                    wins_n[wi] += 1
                    placed = True
                    break
            if not placed:
                win_of[d] = len(wins_used)
                rank_of[d] = 0
                wins_used.append(cd.copy())
                wins_n.append(1)
        packs.append((m, win_of, rank_of))
        W = max(W, len(wins_used))
    W = ((W + GW - 1) // GW) * GW
    COLS = WCOLS * W
    NT = COLS // P
    assert 3 * COLS < 2 ** 15, "L2 table third exceeds int16 range"
    n_grp = W // GW

    # column & global row of every node
    col_all = np.empty(N, np.int64)
    g_row = np.empty(N, np.int64)
    for c in range(n_cores):
        m, win_of, rank_of = packs[c]
        col = win_of * WCOLS + rank_of
        col_all[c * n_local:(c + 1) * n_local] = col
        g_row[c * n_local:(c + 1) * n_local] = \
            c * COLS + (col % P) * NT + col // P

    # ---- per-core edge streams ----
    i1_l, s1_l, i2_l, s2_l = [], [], [], []
    for c in range(n_cores):
        m, win_of, rank_of = packs[c]
        es, ew = src[m], edge_weight[m]
        ed = dst[m] - c * n_local
        ewin = win_of[ed]
        erank = rank_of[ed]
        half1 = (es >= NH).astype(np.int64)
        third = np.searchsorted(np.array(tb[1:3]), es, side="right")

        def build(nsub, sub, base_vals, n_blocks_per_grp):
            # block of edge = grp*(nsub*GW) + sub*GW + (win % GW)
            grp = ewin // GW
            blk = grp * (nsub * GW) + sub * GW + (ewin % GW)
            nblk = n_grp * nsub * GW
            # slot within block: stable order by (blk), cumcount
            order_e = np.lexsort((np.arange(len(es)), blk))
            bsort = blk[order_e]
            first = np.concatenate([[True], bsort[1:] != bsort[:-1]])
            start_pos = np.maximum.accumulate(
                np.where(first, np.arange(len(es)), 0))
            slot_sorted = np.arange(len(es)) - start_pos
            slot = np.empty(len(es), np.int64)
            slot[order_e] = slot_sorted
            assert slot.max(initial=0) < P
            idx = np.zeros((nblk, P), np.int64)
            S = np.zeros((nblk, P, WCOLS), np.float32)
            idx[blk, slot] = base_vals
            S[blk, slot, erank] = ew
            return idx, S

        sub1 = half1
        base1 = np.where(es < NH, es, es - NH)
        idx1, S1 = build(2, sub1, base1, 2 * GW)
        base2 = g_row[es] - np.array([0, tb[1] // n_local * COLS,
                                      tb[2] // n_local * COLS])[third]
        idx2, S2 = build(3, third, base2, 3 * GW)

        # wrapped per-call idx [ncalls, 128, GW*P/16], S [ncalls, 128, GW, 16]
        def to_calls(idx, S, nsub):
            ncalls = n_grp * nsub
            iw = np.empty((ncalls, P, GW * P // 16), np.int16)
            sw = np.empty((ncalls, P, GW, WCOLS), ml_dtypes.bfloat16)
            for call in range(ncalls):
                blocks = idx[call * GW:(call + 1) * GW]      # [GW, P]
                stream = blocks.reshape(GW * P)              # pos j*128+p -> block j slot p
                iw[call] = _wrap_idx(stream)
                sblk = S[call * GW:(call + 1) * GW]          # [GW, P, 16]
                sw[call] = sblk.transpose(1, 0, 2).astype(ml_dtypes.bfloat16)
            return iw, sw

        iw1, sw1 = to_calls(idx1, S1, 2)
        iw2, sw2 = to_calls(idx2, S2, 3)
        i1_l.append(iw1)
        s1_l.append(sw1)
        i2_l.append(iw2)
        s2_l.append(sw2)

    # ---- queries: gather u at owner(qs), AllToAll to owner(qd) ----
    q_owner_s = qs // n_local
    q_owner_d = qd // n_local
    counts = np.zeros((n_cores, n_cores), np.int64)
    np.add.at(counts, (q_owner_s, q_owner_d), 1)
    QSLOT = ((int(counts.max()) + P - 1) // P) * P
    QTOT = n_cores * QSLOT
    QJ = QTOT // P
    loc_row = (col_all % P) * NT + col_all // P  # local table row of node

    qu_l, qv_l = [], []
    send_pos = np.empty(Q, np.int64)   # position in sender's stream
    for c in range(n_cores):
        mine = np.nonzero(q_owner_s == c)[0]
        dests = q_owner_d[mine]
        order = np.argsort(dests, kind="stable")
        mine = mine[order]
        dests = dests[order]
        qu = np.zeros(QTOT, np.int64)
        fill = np.zeros(n_cores, np.int64)
        pos = np.empty(len(mine), np.int64)
        for ii, (q, d) in enumerate(zip(mine, dests)):
            pos[ii] = d * QSLOT + fill[d]
            fill[d] += 1
        qu[pos] = loc_row[qs[mine]]
        send_pos[mine] = pos
        qu_l.append(_wrap_idx(qu))
    # receiver side: position in a2a_out = s*QSLOT + slot
    qv_l = []
    out_map = []  # per core: array [QTOT] of orig query index or -1
    for c in range(n_cores):
        qv = np.zeros(QTOT, np.int64)
        omap = np.full(QTOT, -1, np.int64)
        for s in range(n_cores):
            sel = np.nonzero((q_owner_s == s) & (q_owner_d == c))[0]
            # slots assigned in sender order
            slots = send_pos[sel] - c * QSLOT  # slot within bucket
            qv[s * QSLOT + slots] = loc_row[qd[sel]]
            omap[s * QSLOT + slots] = sel
        qv_l.append(_wrap_idx(qv))
        out_map.append(omap)

    # ---- weights / constants ----
    AB = np.concatenate([W2 @ Wl[:256], W2 @ Wl[256:]], axis=1)  # [256,4]
    cu = b2 @ Wl[:256]
    cv = b2 @ Wl[256:] + bl
    cuv = np.concatenate([cu, cv]).reshape(4, 1).astype(np.float32)
    b1c = b1.reshape(2, P).astype(np.float32)
    x_bf = x.astype(ml_dtypes.bfloat16)
    w1_f = np.ascontiguousarray(W1.astype(np.float32))
    ab_f = np.ascontiguousarray(AB.astype(np.float32))

    in_maps = []
    for c in range(n_cores):
        in_maps.append({
            "x": x_bf, "i1": i1_l[c], "s1": s1_l[c],
            "i2": i2_l[c], "s2": s2_l[c],
            "qu": qu_l[c], "qv": qv_l[c],
            "w1": w1_f, "ab": ab_f, "b1": b1c, "cuv": cuv,
        })

    dims = dict(N=N, NH=NH, W=W, COLS=COLS, NT=NT, QJ=QJ, QSLOT=QSLOT,
                n_grp=n_grp, n_cores=n_cores,
                tsplit=(tb[1] // n_local, tb[2] // n_local))
    if verbose:
        fill1 = E / (n_cores * n_grp * 2 * GW * P)
        fill2 = E / (n_cores * n_grp * 3 * GW * P)
        print(f"plan: W={W} COLS={COLS} NT={NT} QSLOT={QSLOT} QJ={QJ} "
              f"fill1={fill1:.3f} fill2={fill2:.3f}")
    meta = dict(out_map=out_map, Q=Q, QJ=QJ)
    return dims, in_maps, meta


def unshard(results, meta):
    Q, QJ = meta["Q"], meta["QJ"]
    out = np.empty((Q, 2), np.float32)
    for c, res in enumerate(results):
        o = res["out"].reshape(P * QJ, 2)
        omap = meta["out_map"][c]
        # out rows: position pi lives at (p=pi%128, j=pi//128) -> flat p*QJ+j
        pi = np.nonzero(omap >= 0)[0]
        out[omap[pi]] = o[(pi % P) * QJ + pi // P]
    return out


# ----------------------------------------------------------------------------
# Device graph
# ----------------------------------------------------------------------------

def build_nc(dims):
    n_cores = dims["n_cores"]
    N, NH, COLS, NT, QJ = (dims["N"], dims["NH"], dims["COLS"], dims["NT"],
                           dims["QJ"])
    n_grp = dims["n_grp"]
    QTOT = QJ * P

    nc = bacc.Bacc("TRN2", target_bir_lowering=False, debug=False,
                   enable_asserts=False, num_devices=n_cores)

    IW = GW * P // 16
    t_x = nc.dram_tensor("x", [N, 256], BF16, kind="ExternalInput")
    t_i1 = nc.dram_tensor("i1", [n_grp * 2, P, IW], I16, kind="ExternalInput")
    t_s1 = nc.dram_tensor("s1", [n_grp * 2, P, GW, WCOLS], BF16,
                          kind="ExternalInput")
    t_i2 = nc.dram_tensor("i2", [n_grp * 3, P, IW], I16, kind="ExternalInput")
    t_s2 = nc.dram_tensor("s2", [n_grp * 3, P, GW, WCOLS], BF16,
                          kind="ExternalInput")
    t_qu = nc.dram_tensor("qu", [P, QTOT // 16], I16, kind="ExternalInput")
    t_qv = nc.dram_tensor("qv", [P, QTOT // 16], I16, kind="ExternalInput")
    t_w1 = nc.dram_tensor("w1", [256, 256], F32, kind="ExternalInput")
    t_ab = nc.dram_tensor("ab", [256, 4], F32, kind="ExternalInput")
    t_b1 = nc.dram_tensor("b1", [2, P], F32, kind="ExternalInput")
    t_cuv = nc.dram_tensor("cuv", [4, 1], F32, kind="ExternalInput")
    t_out = nc.dram_tensor("out", [P, QJ, 2], F32, kind="ExternalOutput")

    t_yab = nc.dram_tensor("yab_l", [P, NT * 4], BF16)
    t_uvc = nc.dram_tensor("uvc", [n_cores * P, NT * 4], BF16,
                           addr_space="Shared")
    t_uvp = nc.dram_tensor("uvp", [n_cores * COLS, P], BF16)
    t_upad = nc.dram_tensor("upad", [COLS, 64], F32)
    t_vpad = nc.dram_tensor("vpad", [COLS, 64], F32)
    t_a2i = nc.dram_tensor("a2i", [QTOT, 2], F32)
    t_a2o = nc.dram_tensor("a2o", [QTOT, 2], F32)

    tensors = locals()
    with tile.TileContext(nc) as tc:
        with ExitStack() as ctx:
            _emit(ctx, tc, dims, tensors)
    nc.compile()
    return nc


def _emit(ctx, tc, dims, T):
    nc = tc.nc
    n_cores = dims["n_cores"]
    N, NH, COLS, NT, QJ = (dims["N"], dims["NH"], dims["COLS"], dims["NT"],
                           dims["QJ"])
    n_grp = dims["n_grp"]
    c0, c1 = dims["tsplit"]
    QTOT = QJ * P
    IW = GW * P // 16
    NI = GW * P
    Relu = mybir.ActivationFunctionType.Relu
    Copy = mybir.ActivationFunctionType.Copy
    Exp = mybir.ActivationFunctionType.Exp
    Ln = mybir.ActivationFunctionType.Ln

    const = ctx.enter_context(tc.tile_pool(name="const", bufs=1))

    w1A = const.tile([P, 256], F32)
    nc.sync.dma_start(w1A[:], T["t_w1"].ap()[0:P, :])
    w1B = const.tile([P, 256], F32)
    nc.sync.dma_start(w1B[:], T["t_w1"].ap()[P:256, :])
    w1Ar = const.tile([P, 256], F32R)
    nc.vector.tensor_copy(w1Ar[:], w1A[:])
    w1Br = const.tile([P, 256], F32R)
    nc.vector.tensor_copy(w1Br[:], w1B[:])
    abA = const.tile([P, 4], F32)
    nc.sync.dma_start(abA[:], T["t_ab"].ap()[0:P, :])
    abB = const.tile([P, 4], F32)
    nc.sync.dma_start(abB[:], T["t_ab"].ap()[P:256, :])
    b1A = const.tile([P, 1], F32)
    nc.sync.dma_start(b1A[:], T["t_b1"].ap()[0, :, None])
    b1B = const.tile([P, 1], F32)
    nc.sync.dma_start(b1B[:], T["t_b1"].ap()[1, :, None])
    cuv = const.tile([4, 1], F32)
    nc.sync.dma_start(cuv[:], T["t_cuv"].ap()[:, :])
    qu = const.tile([P, QTOT // 16], I16)
    nc.sync.dma_start(qu[:], T["t_qu"].ap()[:, :])
    qv = const.tile([P, QTOT // 16], I16)
    nc.sync.dma_start(qv[:], T["t_qv"].ap()[:, :])
    id4 = const.tile([4, 4], F32)
    make_identity(nc, id4[:])

    # long-lived tail tiles (before h1p: pool closes stay LIFO)
    tail = ctx.enter_context(tc.tile_pool(name="tail", bufs=1))
    ystage = tail.tile([P, NT * 4], BF16)
    uvT = tail.tile([4, COLS], F32)
    uvn = tail.tile([P, NT, 4], F32)

    h1pool_cm = tc.tile_pool(name="h1p", bufs=1)
    h1pool = h1pool_cm.__enter__()
    h1A = h1pool.tile([P, COLS], F32)
    h1B = h1pool.tile([P, COLS], F32)

    x_views = [T["t_x"].ap()[0:NH, :], T["t_x"].ap()[NH:N, :]]

    # ---------------- layer 1 ----------------
    with tc.tile_pool(name="msgs", bufs=2) as msgs_pool, \
         tc.tile_pool(name="idxp", bufs=2) as idxp, \
         tc.tile_pool(name="sp", bufs=2) as sp, \
         tc.tile_pool(name="aggp", bufs=3) as aggp, \
         tc.tile_pool(name="ps1", bufs=2, space="PSUM") as ps1, \
         tc.tile_pool(name="ps1b", bufs=2, space="PSUM") as ps1b, \
         tc.tile_pool(name="psz", bufs=2, space="PSUM") as psz:
        for g in range(n_grp):
            agA = aggp.tile([P, GW * WCOLS], F32R, tag="agA")
            agB = aggp.tile([P, GW * WCOLS], F32R, tag="agB")
            for half in range(2):
                call = g * 2 + half
                idxt = idxp.tile([P, IW], I16, tag="ix")
                nc.sync.dma_start(idxt[:], T["t_i1"].ap()[call, :, :])
                st = sp.tile([P, GW, WCOLS], BF16, tag="s")
                nc.sync.dma_start(st[:], T["t_s1"].ap()[call, :, :, :])
                mts = []
                for s in range(NI // 1024):
                    mt = msgs_pool.tile([P, 8, 256], BF16, tag=f"m1_{s}")
                    nc.gpsimd.dma_gather(
                        mt[:], x_views[half],
                        idxt[:, 64 * s:64 * (s + 1)], 1024, 1024, 256,
                        single_packet=False)
                    mts.append(mt)
                pA = ps1.tile([P, GW * WCOLS], F32, tag="pA")
                pB = ps1b.tile([P, GW * WCOLS], F32, tag="pB")
                for j in range(GW):
                    cs = slice(WCOLS * j, WCOLS * (j + 1))
                    mt = mts[j // 8]
                    jj = j % 8
                    nc.tensor.matmul(pA[:, cs], lhsT=mt[:, jj, 0:P],
                                     rhs=st[:, j, :],
                                     start=(j == 0), stop=(j == GW - 1))
                    nc.tensor.matmul(pB[:, cs], lhsT=mt[:, jj, P:256],
                                     rhs=st[:, j, :],
                                     start=(j == 0), stop=(j == GW - 1))
                if half == 0:
                    nc.scalar.activation(agA[:], pA[:], Copy)
                    nc.vector.tensor_copy(agB[:], pB[:])
                else:
                    nc.vector.tensor_tensor(agA[:], agA[:], pA[:],
                                            op=mybir.AluOpType.add)
                    nc.vector.tensor_tensor(agB[:], agB[:], pB[:],
                                            op=mybir.AluOpType.add)
            cols = slice(g * GW * WCOLS, (g + 1) * GW * WCOLS)
            for m in range(2):
                pz = psz.tile([P, GW * WCOLS], F32, tag="pz")
                nc.tensor.matmul(pz[:], lhsT=w1Ar[:, m * P:(m + 1) * P],
                                 rhs=agA[:], start=True, stop=False)
                nc.tensor.matmul(pz[:], lhsT=w1Br[:, m * P:(m + 1) * P],
                                 rhs=agB[:], start=False, stop=True)
                h1m = h1A if m == 0 else h1B
                b1m = b1A if m == 0 else b1B
                nc.scalar.activation(h1m[:, cols], pz[:], Relu,
                                     bias=b1m[:, 0:1])

    # ---------------- yab = h1 @ [A|B] ----------------
    with tc.tile_pool(name="psy", bufs=1, space="PSUM") as psy:
        py = psy.tile([P, NT * 4], F32)
        for t in range(NT):
            nc.tensor.matmul(py[:, 4 * t:4 * t + 4],
                             lhsT=h1A[:, t * P:(t + 1) * P], rhs=abA[:],
                             start=(t == 0), stop=False)
            nc.tensor.matmul(py[:, 4 * t:4 * t + 4],
                             lhsT=h1B[:, t * P:(t + 1) * P], rhs=abB[:],
                             start=False, stop=(t == NT - 1))
        nc.vector.tensor_copy(ystage[:], py[:])
    nc.sync.dma_start(T["t_yab"].ap()[:, :], ystage[:])
    h1pool_cm.__exit__(None, None, None)

    # ---------------- AllGather yab + pad-spray ----------------
    nc.gpsimd.collective_compute(
        "AllGather", mybir.AluOpType.bypass,
        replica_groups=[list(range(n_cores))],
        ins=[T["t_yab"].ap().opt()],
        outs=[T["t_uvc"].ap().opt()],
    )
    uvc_rows = T["t_uvc"].ap().rearrange("a (b c) -> (a b) c", c=4)
    half_rows = n_cores * COLS // 2
    nc.sync.dma_start(T["t_uvp"].ap()[0:half_rows, 0:4],
                      uvc_rows[0:half_rows, :])
    nc.sync.dma_start(T["t_uvp"].ap()[half_rows:, 0:4],
                      uvc_rows[half_rows:, :])

    third_starts = [0, c0 * COLS, c1 * COLS]
    third_ends = [c0 * COLS, c1 * COLS, n_cores * COLS]
    tuv_views = [T["t_uvp"].ap()[third_starts[t]:third_ends[t], :]
                 for t in range(3)]

    # ---------------- layer 2 ----------------
    with tc.tile_pool(name="m2", bufs=2) as m2pool, \
         tc.tile_pool(name="idxp2", bufs=2) as idxp2, \
         tc.tile_pool(name="sp2", bufs=2) as sp2, \
         tc.tile_pool(name="ps2", bufs=2, space="PSUM") as ps2:
        for g in range(n_grp):
            puv = ps2.tile([4, GW * WCOLS], F32, tag="puv")
            for third in range(3):
                call = g * 3 + third
                idxt = idxp2.tile([P, IW], I16, tag="ix2")
                nc.sync.dma_start(idxt[:], T["t_i2"].ap()[call, :, :])
                st = sp2.tile([P, GW, WCOLS], BF16, tag="s2")
                nc.sync.dma_start(st[:], T["t_s2"].ap()[call, :, :, :])
                mt2s = []
                for s in range(NI // 1024):
                    mt2 = m2pool.tile([P, 8, P], BF16, tag=f"m2_{s}")
                    nc.gpsimd.dma_gather(
                        mt2[:], tuv_views[third],
                        idxt[:, 64 * s:64 * (s + 1)], 1024, 1024, P,
                        single_packet=False)
                    mt2s.append(mt2)
                for j in range(GW):
                    cs = slice(WCOLS * j, WCOLS * (j + 1))
                    nc.tensor.matmul(puv[:, cs], lhsT=mt2s[j // 8][:, j % 8, 0:4],
                                     rhs=st[:, j, :],
                                     start=(third == 0 and j == 0),
                                     stop=(third == 2 and j == GW - 1))
            nc.vector.tensor_tensor(
                uvT[:, g * GW * WCOLS:(g + 1) * GW * WCOLS], puv[:],
                cuv[:, 0:1].to_broadcast([4, GW * WCOLS]),
                op=mybir.AluOpType.add)

    # ---------------- transpose uvT -> node-major, build u/v tables -------
    with tc.tile_pool(name="pst", bufs=2, space="PSUM") as pst:
        for t in range(NT):
            ptp = pst.tile([P, 4], F32, tag="ptp")
            nc.tensor.transpose(ptp[:], uvT[:, t * P:(t + 1) * P], id4[:])
            nc.vector.tensor_copy(uvn[:, t, :], ptp[:])
    upad_rows = T["t_upad"].ap()[:, 0:2].rearrange("(p t) c -> p t c", p=P)
    vpad_rows = T["t_vpad"].ap()[:, 0:2].rearrange("(p t) c -> p t c", p=P)
    nc.sync.dma_start(upad_rows, uvn[:, :, 0:2])
    nc.sync.dma_start(vpad_rows, uvn[:, :, 2:4])

    # ---------------- query head ----------------
    qp = ctx.enter_context(tc.tile_pool(name="qp", bufs=1))
    ug = qp.tile([P, QJ, 64], F32)
    for s in range(QTOT // 1024):
        nc.gpsimd.dma_gather(
            ug[:, 8 * s:8 * (s + 1), :], T["t_upad"].ap()[:, :],
            qu[:, 64 * s:64 * (s + 1)], 1024, 1024, 64,
            single_packet=False)
    us = qp.tile([P, QJ, 2], F32)
    nc.vector.tensor_copy(us[:], ug[:, :, 0:2])
    a2i_v = T["t_a2i"].ap().rearrange("(j p) c -> p j c", p=P)
    nc.sync.dma_start(a2i_v, us[:])
    nc.gpsimd.collective_compute(
        "AllToAll", mybir.AluOpType.bypass,
        replica_groups=[list(range(n_cores))],
        ins=[T["t_a2i"].ap().opt()],
        outs=[T["t_a2o"].ap().opt()],
    )
    vg = qp.tile([P, QJ, 64], F32)
    for s in range(QTOT // 1024):
        nc.gpsimd.dma_gather(
            vg[:, 8 * s:8 * (s + 1), :], T["t_vpad"].ap()[:, :],
            qv[:, 64 * s:64 * (s + 1)], 1024, 1024, 64,
            single_packet=False)
    ut2 = qp.tile([P, QJ, 2], F32)
    a2o_v = T["t_a2o"].ap().rearrange("(j p) c -> p j c", p=P)
    nc.sync.dma_start(ut2[:], a2o_v)

    lg = qp.tile([P, QJ, 2], F32)
    nc.vector.tensor_tensor(lg[:], ut2[:], vg[:, :, 0:2],
                            op=mybir.AluOpType.add)
    mx = qp.tile([P, QJ, 1], F32)
    nc.vector.reduce_max(mx[:], lg[:], axis=mybir.AxisListType.X)
    tt = qp.tile([P, QJ, 2], F32)
    nc.vector.tensor_tensor(tt[:], lg[:], mx[:].to_broadcast([P, QJ, 2]),
                            op=mybir.AluOpType.subtract)
    ex = qp.tile([P, QJ, 2], F32)
    nc.scalar.activation(ex[:], tt[:], Exp)
    sm = qp.tile([P, QJ, 1], F32)
    nc.vector.reduce_sum(sm[:], ex[:], axis=mybir.AxisListType.X)
    ls = qp.tile([P, QJ, 1], F32)
    nc.scalar.activation(ls[:], sm[:], Ln)
    oo = qp.tile([P, QJ, 2], F32)
    nc.vector.tensor_tensor(oo[:], tt[:], ls[:].to_broadcast([P, QJ, 2]),
                            op=mybir.AluOpType.subtract)
    nc.sync.dma_start(T["t_out"].ap()[:, :, :], oo[:])


# ----------------------------------------------------------------------------
# numpy reference (mirrors reference.py math in f32)
# ----------------------------------------------------------------------------

def numpy_reference(inputs):
    x = np.asarray(inputs["x"], np.float32)
    ei = np.asarray(inputs["edge_index"], np.int64)
    qe = np.asarray(inputs["query_edges"], np.int64)
    w = np.asarray(inputs["edge_weight"], np.float32)
    W1, b1 = np.asarray(inputs["W1"], np.float32), np.asarray(inputs["b1"], np.float32)
    W2, b2 = np.asarray(inputs["W2"], np.float32), np.asarray(inputs["b2"], np.float32)
    Wl, bl = np.asarray(inputs["Wl"], np.float32), np.asarray(inputs["bl"], np.float32)
    N = x.shape[0]
    src, dst = ei[0], ei[1]

    def conv(h, W, b):
        z = h @ W
        msg = z[src] * w[:, None]
        agg = np.zeros((N, z.shape[1]), np.float32)
        np.add.at(agg, dst, msg)
        return agg + b

    h1 = np.maximum(conv(x, W1, b1), 0.0)
    h2 = conv(h1, W2, b2)
    q = np.concatenate([h2[qe[:, 0]], h2[qe[:, 1]]], axis=1)
    logits = q @ Wl + bl
    m = logits.max(axis=1, keepdims=True)
    e = np.exp(logits - m)
    return logits - m - np.log(e.sum(axis=1, keepdims=True))

# ----------------------------------------------------------------------------
# Entry point: full inputs in, full output out
# ----------------------------------------------------------------------------

LAST_RESULTS = None


def kernel(**inputs):
    """Takes the FULL (unsharded) inputs of nn_DiGCN_link_prediction and
    returns the full [N_QUERY, 2] float32 log-softmax output.

    Shards nodes/edges/queries across 8 NeuronCores internally, runs one
    SPMD Bass kernel (per-core data, identical graph), and reassembles.
    """
    global LAST_RESULTS
    import os
    from concourse.bass_utils import run_bass_kernel_spmd

    n_cores = 8
    dims, in_maps, meta = plan(inputs, n_cores=n_cores)
    nc = build_nc(dims)
    res = run_bass_kernel_spmd(
        nc, in_maps, core_ids=list(range(n_cores)),
        trace=bool(int(os.environ.get("GNN_TRACE", "0"))),
        stitch_traces=False,
    )
    LAST_RESULTS = res
    return unshard(res.results, meta)



# revision 2
# speedup vs baseline: 1.9449x; 1.9449x over previous
"""DiGCN link prediction on 8 TRN2 NeuronCores.

Math (reference):
    h1 = relu(segsum_dst(w_e * (x@W1)[src]) + b1)
    h2 = segsum_dst(w_e * (h1@W2)[src]) + b2
    logits = concat(h2[qs], h2[qd]) @ Wl + bl ; out = log_softmax(logits)

Device strategy (per core, SPMD-identical graph, per-core data):
  - dst-sharded edges. Host packs each core's dsts into fixed 16-column
    windows (best-fit, lo/hi caps 128), 32 windows per 512-col PSUM group.
  - Layer 1 uses linearity: segsum(w, x@W1) = segsum(w, x)@W1. Per
    (group, half) one 4096-idx dma_gather (src<25000 / >=25000 halves,
    int16 from the two x table halves), per 128-slot block a matmul
    msgs^T[128e,128f] @ S[128e,16] into feature-major PSUM accumulating
    both halves via SBUF adds. Then project W1^T (f32r) + relu + b1.
  - Layer 2 + head use linearity again: with A=W2@Wl[:256], B=W2@Wl[256:],
    u[d]=sum w_e*(h1@A)[src]+b2@Wlt, v[d]=sum w_e*(h1@B)[src]+b2@Wlb+bl,
    logits[q] = u[qs]+v[qd]. yab=h1@[A|B] ([*,4] bf16) is AllGathered
    (0.5 MB) and pad-sprayed into a PAIRED 512B-row table (two nodes per
    row: even node payload at cols [0:4], odd at [128:132]). Each L2
    gather call covers the same (group, half) edge blocks as L1 (the
    pair-table half boundary coincides with the x half boundary), with
    the S matrix split by pair parity (two matmuls per window).
  - Query head: queries are processed where qs lives (local u gather from
    a padded 256B-row u table), the u-halves are AllToAll'd to the qd
    owner, which gathers v locally, adds, and takes log_softmax.
"""

import math
from contextlib import ExitStack

import ml_dtypes
import numpy as np

import concourse.bass as bass
import concourse.tile as tile
from concourse import bacc, mybir
from concourse.masks import make_identity

BF16 = mybir.dt.bfloat16
F32 = mybir.dt.float32
F32R = mybir.dt.float32r
I16 = mybir.dt.int16
P = 128
WCOLS = 16      # columns per window
GW = 32         # windows per PSUM group (512 cols)


def _wrap_idx(stream):
    """[n] int -> [128, n//16] int16 in dma_gather wrapped layout."""
    n = len(stream)
    a = np.asarray(stream, np.int64).reshape(n // 16, 16).T
    return np.tile(a, (8, 1)).astype(np.int16)


def _pack_windows(lo, hi, n_bins, cap):
    """Best-fit-decreasing pack of dsts into <=n_bins windows of <=WCOLS
    dsts with per-window lo/hi sums <= cap. Returns (win_of, rank_of, W)
    or None if infeasible within n_bins."""
    n = len(lo)
    order = np.argsort(-(lo + hi), kind="stable")
    bin_lo = np.zeros(n_bins, np.int64)
    bin_hi = np.zeros(n_bins, np.int64)
    bin_n = np.zeros(n_bins, np.int64)
    win_of = np.empty(n, np.int64)
    rank_of = np.empty(n, np.int64)
    for d in order:
        ld, hd = lo[d], hi[d]
        ok = (bin_n < WCOLS) & (bin_lo + ld <= cap) & (bin_hi + hd <= cap)
        if not ok.any():
            return None
        # best fit: most-loaded feasible bin
        load = np.where(ok, bin_lo + bin_hi, -1)
        b = int(np.argmax(load))
        win_of[d] = b
        rank_of[d] = bin_n[b]
        bin_lo[b] += ld
        bin_hi[b] += hd
        bin_n[b] += 1
    used = int(np.max(win_of)) + 1
    return win_of, rank_of, used


def plan(inputs, n_cores=8, verbose=False):
    x = np.asarray(inputs["x"], np.float32)
    edge_index = np.asarray(inputs["edge_index"], np.int64)
    query_edges = np.asarray(inputs["query_edges"], np.int64)
    edge_weight = np.asarray(inputs["edge_weight"], np.float32)
    W1 = np.asarray(inputs["W1"], np.float32)
    b1 = np.asarray(inputs["b1"], np.float32)
    W2 = np.asarray(inputs["W2"], np.float32)
    b2 = np.asarray(inputs["b2"], np.float32)
    Wl = np.asarray(inputs["Wl"], np.float32)
    bl = np.asarray(inputs["bl"], np.float32)

    N, F = x.shape
    E = edge_index.shape[1]
    Q = query_edges.shape[0]
    assert F == 256 and N % n_cores == 0
    n_local = N // n_cores
    NH = N // 2                      # x table split point (= core 4 boundary)
    assert NH == (n_cores // 2) * n_local

    src = edge_index[0]
    dst = edge_index[1]
    qs, qd = query_edges[:, 0], query_edges[:, 1]

    # ---- pack windows per core (lo/hi caps only) ----
    CAP = 128
    packs = []
    W = 0
    for c in range(n_cores):
        m = dst // n_local == c
        ed = dst[m] - c * n_local
        es = src[m]
        lo1 = np.bincount(ed[es < NH], minlength=n_local)
        hi1 = np.bincount(ed[es >= NH], minlength=n_local)
        assert max(lo1.max(), hi1.max()) <= CAP
        for nb in (16 * GW, 17 * GW, 18 * GW):
            res = _pack_windows(lo1, hi1, nb, CAP)
            if res is not None:
                break
        assert res is not None, "window packing failed"
        win_of, rank_of, used = res
        packs.append((m, win_of, rank_of))
        W = max(W, used)
    W = ((W + GW - 1) // GW) * GW
    COLS = WCOLS * W
    NT = COLS // P
    n_grp = W // GW
    NPAIR = n_cores * COLS // 2       # rows of the paired L2 table
    assert NPAIR // 2 <= 2 ** 15, "L2 pair table half exceeds int16 range"

    # column & global row of every node
    col_all = np.empty(N, np.int64)
    g_row = np.empty(N, np.int64)
    for c in range(n_cores):
        m, win_of, rank_of = packs[c]
        col = win_of * WCOLS + rank_of
        col_all[c * n_local:(c + 1) * n_local] = col
        g_row[c * n_local:(c + 1) * n_local] = \
            c * COLS + (col % P) * NT + col // P

    # ---- per-core edge streams (L1 and L2 share the block structure) ----
    i1_l, s1_l, i2_l, s2_l = [], [], [], []
    for c in range(n_cores):
        m, win_of, rank_of = packs[c]
        es, ew = src[m], edge_weight[m]
        ed = dst[m] - c * n_local
        ewin = win_of[ed]
        erank = rank_of[ed]
        half1 = (es >= NH).astype(np.int64)

        # block of edge = grp*(2*GW) + half*GW + (win % GW)
        grp = ewin // GW
        blk = grp * (2 * GW) + half1 * GW + (ewin % GW)
        nblk = n_grp * 2 * GW
        order_e = np.lexsort((np.arange(len(es)), blk))
        bsort = blk[order_e]
        first = np.concatenate([[True], bsort[1:] != bsort[:-1]])
        start_pos = np.maximum.accumulate(
            np.where(first, np.arange(len(es)), 0))
        slot_sorted = np.arange(len(es)) - start_pos
        slot = np.empty(len(es), np.int64)
        slot[order_e] = slot_sorted
        assert slot.max(initial=0) < P

        base1 = np.where(es < NH, es, es - NH)
        prow = g_row[es] // 2
        par = g_row[es] % 2
        base2 = prow - (NPAIR // 2) * half1
        assert base2.min(initial=0) >= 0 and base2.max(initial=0) < NPAIR // 2

        idx1 = np.zeros((nblk, P), np.int64)
        idx2 = np.zeros((nblk, P), np.int64)
        S1 = np.zeros((nblk, P, WCOLS), np.float32)
        S2 = np.zeros((nblk, P, 2, WCOLS), np.float32)
        idx1[blk, slot] = base1
        idx2[blk, slot] = base2
        S1[blk, slot, erank] = ew
        S2[blk, slot, par, erank] = ew

        # wrapped per-call idx [ncalls, 128, GW*P/16], S per call
        ncalls = n_grp * 2
        iw1 = np.empty((ncalls, P, GW * P // 16), np.int16)
        iw2 = np.empty((ncalls, P, GW * P // 16), np.int16)
        sw1 = np.empty((ncalls, P, GW, WCOLS), ml_dtypes.bfloat16)
        sw2 = np.empty((ncalls, P, GW, 2, WCOLS), ml_dtypes.bfloat16)
        for call in range(ncalls):
            sl = slice(call * GW, (call + 1) * GW)
            iw1[call] = _wrap_idx(idx1[sl].reshape(GW * P))
            iw2[call] = _wrap_idx(idx2[sl].reshape(GW * P))
            sw1[call] = S1[sl].transpose(1, 0, 2).astype(ml_dtypes.bfloat16)
            sw2[call] = S2[sl].transpose(1, 0, 2, 3).astype(ml_dtypes.bfloat16)
        i1_l.append(iw1)
        s1_l.append(sw1)
        i2_l.append(iw2)
        s2_l.append(sw2)

    # ---- queries: gather u at owner(qs), AllToAll to owner(qd) ----
    q_owner_s = qs // n_local
    q_owner_d = qd // n_local
    counts = np.zeros((n_cores, n_cores), np.int64)
    np.add.at(counts, (q_owner_s, q_owner_d), 1)
    QSLOT = ((int(counts.max()) + P - 1) // P) * P
    QTOT = n_cores * QSLOT
    QJ = QTOT // P
    loc_row = (col_all % P) * NT + col_all // P  # local table row of node

    qu_l = []
    send_pos = np.empty(Q, np.int64)   # position in sender's stream
    for c in range(n_cores):
        mine = np.nonzero(q_owner_s == c)[0]
        dests = q_owner_d[mine]
        order = np.argsort(dests, kind="stable")
        mine = mine[order]
        dests = dests[order]
        qu = np.zeros(QTOT, np.int64)
        fill = np.zeros(n_cores, np.int64)
        pos = np.empty(len(mine), np.int64)
        for ii, (q, d) in enumerate(zip(mine, dests)):
            pos[ii] = d * QSLOT + fill[d]
            fill[d] += 1
        qu[pos] = loc_row[qs[mine]]
        send_pos[mine] = pos
        qu_l.append(_wrap_idx(qu))
    # receiver side: position in a2a_out = s*QSLOT + slot
    qv_l = []
    out_map = []  # per core: array [QTOT] of orig query index or -1
    for c in range(n_cores):
        qv = np.zeros(QTOT, np.int64)
        omap = np.full(QTOT, -1, np.int64)
        for s in range(n_cores):
            sel = np.nonzero((q_owner_s == s) & (q_owner_d == c))[0]
            slots = send_pos[sel] - c * QSLOT  # slot within bucket
            qv[s * QSLOT + slots] = loc_row[qd[sel]]
            omap[s * QSLOT + slots] = sel
        qv_l.append(_wrap_idx(qv))
        out_map.append(omap)

    # ---- weights / constants ----
    AB = np.concatenate([W2 @ Wl[:256], W2 @ Wl[256:]], axis=1)  # [256,4]
    cu = b2 @ Wl[:256]
    cv = b2 @ Wl[256:] + bl
    cuv = np.concatenate([cu, cv]).reshape(4, 1).astype(np.float32)
    b1c = b1.reshape(2, P).astype(np.float32)
    x_bf = x.astype(ml_dtypes.bfloat16)
    w1_f = np.ascontiguousarray(W1.astype(np.float32))
    ab_f = np.ascontiguousarray(AB.astype(np.float32))

    in_maps = []
    for c in range(n_cores):
        in_maps.append({
            "x": x_bf, "i1": i1_l[c], "s1": s1_l[c],
            "i2": i2_l[c], "s2": s2_l[c],
            "qu": qu_l[c], "qv": qv_l[c],
            "w1": w1_f, "ab": ab_f, "b1": b1c, "cuv": cuv,
        })

    dims = dict(N=N, NH=NH, W=W, COLS=COLS, NT=NT, QJ=QJ, QSLOT=QSLOT,
                n_grp=n_grp, n_cores=n_cores, NPAIR=NPAIR)
    if verbose:
        fill = E / (n_cores * n_grp * 2 * GW * P)
        print(f"plan: W={W} COLS={COLS} NT={NT} QSLOT={QSLOT} QJ={QJ} "
              f"fill={fill:.3f}")
    meta = dict(out_map=out_map, Q=Q, QJ=QJ)
    return dims, in_maps, meta


def unshard(results, meta):
    Q, QJ = meta["Q"], meta["QJ"]
    out = np.empty((Q, 2), np.float32)
    for c, res in enumerate(results):
        o = res["out"].reshape(P * QJ, 2)
        omap = meta["out_map"][c]
        # out rows: position pi lives at (p=pi%128, j=pi//128) -> flat p*QJ+j
        pi = np.nonzero(omap >= 0)[0]
        out[omap[pi]] = o[(pi % P) * QJ + pi // P]
    return out


# ----------------------------------------------------------------------------
# Device graph
# ----------------------------------------------------------------------------

def build_nc(dims):
    n_cores = dims["n_cores"]
    N, NH, COLS, NT, QJ = (dims["N"], dims["NH"], dims["COLS"], dims["NT"],
                           dims["QJ"])
    n_grp = dims["n_grp"]
    NPAIR = dims["NPAIR"]
    QTOT = QJ * P

    nc = bacc.Bacc("TRN2", target_bir_lowering=False, debug=False,
                   enable_asserts=False, num_devices=n_cores)

    IW = GW * P // 16
    t_x = nc.dram_tensor("x", [N, 256], BF16, kind="ExternalInput")
    t_i1 = nc.dram_tensor("i1", [n_grp * 2, P, IW], I16, kind="ExternalInput")
    t_s1 = nc.dram_tensor("s1", [n_grp * 2, P, GW, WCOLS], BF16,
                          kind="ExternalInput")
    t_i2 = nc.dram_tensor("i2", [n_grp * 2, P, IW], I16, kind="ExternalInput")
    t_s2 = nc.dram_tensor("s2", [n_grp * 2, P, GW, 2, WCOLS], BF16,
                          kind="ExternalInput")
    t_qu = nc.dram_tensor("qu", [P, QTOT // 16], I16, kind="ExternalInput")
    t_qv = nc.dram_tensor("qv", [P, QTOT // 16], I16, kind="ExternalInput")
    t_w1 = nc.dram_tensor("w1", [256, 256], F32, kind="ExternalInput")
    t_ab = nc.dram_tensor("ab", [256, 4], F32, kind="ExternalInput")
    t_b1 = nc.dram_tensor("b1", [2, P], F32, kind="ExternalInput")
    t_cuv = nc.dram_tensor("cuv", [4, 1], F32, kind="ExternalInput")
    t_out = nc.dram_tensor("out", [P, QJ, 2], F32, kind="ExternalOutput")

    t_yab = nc.dram_tensor("yab_l", [P, NT * 4], BF16)
    t_uvc = nc.dram_tensor("uvc", [n_cores * P, NT * 4], BF16,
                           addr_space="Shared")
    t_uvp = nc.dram_tensor("uvp", [NPAIR, 256], BF16)
    t_upad = nc.dram_tensor("upad", [COLS, 64], F32)
    t_vpad = nc.dram_tensor("vpad", [COLS, 64], F32)
    t_a2i = nc.dram_tensor("a2i", [QTOT, 2], F32)
    t_a2o = nc.dram_tensor("a2o", [QTOT, 2], F32)

    tensors = locals()
    with tile.TileContext(nc) as tc:
        with ExitStack() as ctx:
            _emit(ctx, tc, dims, tensors)
    nc.compile()
    return nc


def _emit(ctx, tc, dims, T):
    nc = tc.nc
    n_cores = dims["n_cores"]
    N, NH, COLS, NT, QJ = (dims["N"], dims["NH"], dims["COLS"], dims["NT"],
                           dims["QJ"])
    n_grp = dims["n_grp"]
    NPAIR = dims["NPAIR"]
    QTOT = QJ * P
    IW = GW * P // 16
    NI = GW * P
    Relu = mybir.ActivationFunctionType.Relu
    Copy = mybir.ActivationFunctionType.Copy
    Exp = mybir.ActivationFunctionType.Exp
    Ln = mybir.ActivationFunctionType.Ln

    const = ctx.enter_context(tc.tile_pool(name="const", bufs=1))

    w1A = const.tile([P, 256], F32)
    nc.sync.dma_start(w1A[:], T["t_w1"].ap()[0:P, :])
    w1B = const.tile([P, 256], F32)
    nc.sync.dma_start(w1B[:], T["t_w1"].ap()[P:256, :])
    w1Ar = const.tile([P, 256], F32R)
    nc.vector.tensor_copy(w1Ar[:], w1A[:])
    w1Br = const.tile([P, 256], F32R)
    nc.vector.tensor_copy(w1Br[:], w1B[:])
    abA = const.tile([P, 4], F32)
    nc.sync.dma_start(abA[:], T["t_ab"].ap()[0:P, :])
    abB = const.tile([P, 4], F32)
    nc.sync.dma_start(abB[:], T["t_ab"].ap()[P:256, :])
    b1A = const.tile([P, 1], F32)
    nc.sync.dma_start(b1A[:], T["t_b1"].ap()[0, :, None])
    b1B = const.tile([P, 1], F32)
    nc.sync.dma_start(b1B[:], T["t_b1"].ap()[1, :, None])
    cuv = const.tile([4, 1], F32)
    nc.sync.dma_start(cuv[:], T["t_cuv"].ap()[:, :])
    qu = const.tile([P, QTOT // 16], I16)
    nc.sync.dma_start(qu[:], T["t_qu"].ap()[:, :])
    qv = const.tile([P, QTOT // 16], I16)
    nc.sync.dma_start(qv[:], T["t_qv"].ap()[:, :])
    id4 = const.tile([4, 4], F32)
    make_identity(nc, id4[:])

    # long-lived tail tiles (before h1p: pool closes stay LIFO)
    tail = ctx.enter_context(tc.tile_pool(name="tail", bufs=1))
    ystage = tail.tile([P, NT * 4], BF16)
    uvT = tail.tile([4, COLS], F32)
    uvn = tail.tile([P, NT, 4], F32)

    h1pool_cm = tc.tile_pool(name="h1p", bufs=1)
    h1pool = h1pool_cm.__enter__()
    h1A = h1pool.tile([P, COLS], F32)
    h1B = h1pool.tile([P, COLS], F32)

    x_views = [T["t_x"].ap()[0:NH, :], T["t_x"].ap()[NH:N, :]]

    # ---------------- layer 1 ----------------
    with tc.tile_pool(name="msgs", bufs=2) as msgs_pool, \
         tc.tile_pool(name="idxp", bufs=2) as idxp, \
         tc.tile_pool(name="sp", bufs=2) as sp, \
         tc.tile_pool(name="aggp", bufs=3) as aggp, \
         tc.tile_pool(name="ps1", bufs=2, space="PSUM") as ps1, \
         tc.tile_pool(name="ps1b", bufs=2, space="PSUM") as ps1b, \
         tc.tile_pool(name="psz", bufs=2, space="PSUM") as psz:
        for g in range(n_grp):
            agA = aggp.tile([P, GW * WCOLS], F32R, tag="agA")
            agB = aggp.tile([P, GW * WCOLS], F32R, tag="agB")
            for half in range(2):
                call = g * 2 + half
                idxt = idxp.tile([P, IW], I16, tag="ix")
                nc.sync.dma_start(idxt[:], T["t_i1"].ap()[call, :, :])
                st = sp.tile([P, GW, WCOLS], BF16, tag="s")
                nc.sync.dma_start(st[:], T["t_s1"].ap()[call, :, :, :])
                mt = msgs_pool.tile([P, GW, 256], BF16, tag="m1")
                nc.gpsimd.dma_gather(
                    mt[:], x_views[half], idxt[:, :], NI, NI, 256,
                    single_packet=False)
                pA = ps1.tile([P, GW * WCOLS], F32, tag="pA")
                pB = ps1b.tile([P, GW * WCOLS], F32, tag="pB")
                for j in range(GW):
                    cs = slice(WCOLS * j, WCOLS * (j + 1))
                    nc.tensor.matmul(pA[:, cs], lhsT=mt[:, j, 0:P],
                                     rhs=st[:, j, :],
                                     start=(j == 0), stop=(j == GW - 1))
                    nc.tensor.matmul(pB[:, cs], lhsT=mt[:, j, P:256],
                                     rhs=st[:, j, :],
                                     start=(j == 0), stop=(j == GW - 1))
                if half == 0:
                    nc.scalar.activation(agA[:], pA[:], Copy)
                    nc.vector.tensor_copy(agB[:], pB[:])
                else:
                    nc.vector.tensor_tensor(agA[:], agA[:], pA[:],
                                            op=mybir.AluOpType.add)
                    nc.vector.tensor_tensor(agB[:], agB[:], pB[:],
                                            op=mybir.AluOpType.add)
            cols = slice(g * GW * WCOLS, (g + 1) * GW * WCOLS)
            for m in range(2):
                pz = psz.tile([P, GW * WCOLS], F32, tag="pz")
                nc.tensor.matmul(pz[:], lhsT=w1Ar[:, m * P:(m + 1) * P],
                                 rhs=agA[:], start=True, stop=False)
                nc.tensor.matmul(pz[:], lhsT=w1Br[:, m * P:(m + 1) * P],
                                 rhs=agB[:], start=False, stop=True)
                h1m = h1A if m == 0 else h1B
                b1m = b1A if m == 0 else b1B
                nc.scalar.activation(h1m[:, cols], pz[:], Relu,
                                     bias=b1m[:, 0:1])

    # ---------------- yab = h1 @ [A|B] ----------------
    with tc.tile_pool(name="psy", bufs=1, space="PSUM") as psy:
        py = psy.tile([P, NT * 4], F32)
        for t in range(NT):
            nc.tensor.matmul(py[:, 4 * t:4 * t + 4],
                             lhsT=h1A[:, t * P:(t + 1) * P], rhs=abA[:],
                             start=(t == 0), stop=False)
            nc.tensor.matmul(py[:, 4 * t:4 * t + 4],
                             lhsT=h1B[:, t * P:(t + 1) * P], rhs=abB[:],
                             start=False, stop=(t == NT - 1))
        nc.vector.tensor_copy(ystage[:], py[:])
    nc.sync.dma_start(T["t_yab"].ap()[:, :], ystage[:])
    h1pool_cm.__exit__(None, None, None)

    # ---------------- AllGather yab + paired pad-spray ----------------
    nc.gpsimd.collective_compute(
        "AllGather", mybir.AluOpType.bypass,
        replica_groups=[list(range(n_cores))],
        ins=[T["t_yab"].ap().opt()],
        outs=[T["t_uvc"].ap().opt()],
    )
    # uvc as [pair, 2, 4]: even member -> uvp[:,0:4], odd -> uvp[:,128:132]
    uvc_pairs = T["t_uvc"].ap().rearrange("a (b two c) -> (a b) two c",
                                          two=2, c=4)
    half_rows = NPAIR // 2
    nc.sync.dma_start(T["t_uvp"].ap()[0:half_rows, 0:4],
                      uvc_pairs[0:half_rows, 0, :])
    nc.sync.dma_start(T["t_uvp"].ap()[half_rows:, 0:4],
                      uvc_pairs[half_rows:, 0, :])
    nc.sync.dma_start(T["t_uvp"].ap()[0:half_rows, 128:132],
                      uvc_pairs[0:half_rows, 1, :])
    nc.sync.dma_start(T["t_uvp"].ap()[half_rows:, 128:132],
                      uvc_pairs[half_rows:, 1, :])

    uvp_views = [T["t_uvp"].ap()[0:half_rows, :],
                 T["t_uvp"].ap()[half_rows:NPAIR, :]]

    # ---------------- layer 2 ----------------
    with tc.tile_pool(name="m2", bufs=2) as m2pool, \
         tc.tile_pool(name="idxp2", bufs=2) as idxp2, \
         tc.tile_pool(name="sp2", bufs=2) as sp2, \
         tc.tile_pool(name="ps2", bufs=2, space="PSUM") as ps2:
        for g in range(n_grp):
            puv = ps2.tile([4, GW * WCOLS], F32, tag="puv")
            for half in range(2):
                call = g * 2 + half
                idxt = idxp2.tile([P, IW], I16, tag="ix2")
                nc.sync.dma_start(idxt[:], T["t_i2"].ap()[call, :, :])
                st = sp2.tile([P, GW, 2, WCOLS], BF16, tag="s2")
                nc.sync.dma_start(st[:], T["t_s2"].ap()[call, :, :, :, :])
                mt2 = m2pool.tile([P, GW, 256], BF16, tag="m2")
                nc.gpsimd.dma_gather(
                    mt2[:], uvp_views[half], idxt[:, :], NI, NI, 256,
                    single_packet=False)
                for j in range(GW):
                    cs = slice(WCOLS * j, WCOLS * (j + 1))
                    for par in range(2):
                        nc.tensor.matmul(
                            puv[:, cs],
                            lhsT=mt2[:, j, 128 * par:128 * par + 4],
                            rhs=st[:, j, par, :],
                            start=(half == 0 and j == 0 and par == 0),
                            stop=(half == 1 and j == GW - 1 and par == 1))
            nc.vector.tensor_tensor(
                uvT[:, g * GW * WCOLS:(g + 1) * GW * WCOLS], puv[:],
                cuv[:, 0:1].to_broadcast([4, GW * WCOLS]),
                op=mybir.AluOpType.add)

    # ---------------- transpose uvT -> node-major, build u/v tables -------
    with tc.tile_pool(name="pst", bufs=2, space="PSUM") as pst:
        for t in range(NT):
            ptp = pst.tile([P, 4], F32, tag="ptp")
            nc.tensor.transpose(ptp[:], uvT[:, t * P:(t + 1) * P], id4[:])
            nc.vector.tensor_copy(uvn[:, t, :], ptp[:])
    upad_rows = T["t_upad"].ap()[:, 0:2].rearrange("(p t) c -> p t c", p=P)
    vpad_rows = T["t_vpad"].ap()[:, 0:2].rearrange("(p t) c -> p t c", p=P)
    nc.sync.dma_start(upad_rows, uvn[:, :, 0:2])
    nc.sync.dma_start(vpad_rows, uvn[:, :, 2:4])

    # ---------------- query head ----------------
    qp = ctx.enter_context(tc.tile_pool(name="qp", bufs=1))
    ug = qp.tile([P, QJ, 64], F32)
    nc.gpsimd.dma_gather(
        ug[:], T["t_upad"].ap()[:, :], qu[:, :], QTOT, QTOT, 64,
        single_packet=False)
    us = qp.tile([P, QJ, 2], F32)
    nc.vector.tensor_copy(us[:], ug[:, :, 0:2])
    a2i_v = T["t_a2i"].ap().rearrange("(j p) c -> p j c", p=P)
    nc.sync.dma_start(a2i_v, us[:])
    nc.gpsimd.collective_compute(
        "AllToAll", mybir.AluOpType.bypass,
        replica_groups=[list(range(n_cores))],
        ins=[T["t_a2i"].ap().opt()],
        outs=[T["t_a2o"].ap().opt()],
    )
    vg = qp.tile([P, QJ, 64], F32)
    nc.gpsimd.dma_gather(
        vg[:], T["t_vpad"].ap()[:, :], qv[:, :], QTOT, QTOT, 64,
        single_packet=False)
    ut2 = qp.tile([P, QJ, 2], F32)
    a2o_v = T["t_a2o"].ap().rearrange("(j p) c -> p j c", p=P)
    nc.sync.dma_start(ut2[:], a2o_v)

    lg = qp.tile([P, QJ, 2], F32)
    nc.vector.tensor_tensor(lg[:], ut2[:], vg[:, :, 0:2],
                            op=mybir.AluOpType.add)
    mx = qp.tile([P, QJ, 1], F32)
    nc.vector.reduce_max(mx[:], lg[:], axis=mybir.AxisListType.X)
    tt = qp.tile([P, QJ, 2], F32)
    nc.vector.tensor_tensor(tt[:], lg[:], mx[:].to_broadcast([P, QJ, 2]),
                            op=mybir.AluOpType.subtract)
    ex = qp.tile([P, QJ, 2], F32)
    nc.scalar.activation(ex[:], tt[:], Exp)
    sm = qp.tile([P, QJ, 1], F32)
    nc.vector.reduce_sum(sm[:], ex[:], axis=mybir.AxisListType.X)
    ls = qp.tile([P, QJ, 1], F32)
    nc.scalar.activation(ls[:], sm[:], Ln)
    oo = qp.tile([P, QJ, 2], F32)
    nc.vector.tensor_tensor(oo[:], tt[:], ls[:].to_broadcast([P, QJ, 2]),
                            op=mybir.AluOpType.subtract)
    nc.sync.dma_start(T["t_out"].ap()[:, :, :], oo[:])


# ----------------------------------------------------------------------------
# numpy reference (mirrors reference.py math in f32)
# ----------------------------------------------------------------------------

def numpy_reference(inputs):
    x = np.asarray(inputs["x"], np.float32)
    ei = np.asarray(inputs["edge_index"], np.int64)
    qe = np.asarray(inputs["query_edges"], np.int64)
    w = np.asarray(inputs["edge_weight"], np.float32)
    W1, b1 = np.asarray(inputs["W1"], np.float32), np.asarray(inputs["b1"], np.float32)
    W2, b2 = np.asarray(inputs["W2"], np.float32), np.asarray(inputs["b2"], np.float32)
    Wl, bl = np.asarray(inputs["Wl"], np.float32), np.asarray(inputs["bl"], np.float32)
    N = x.shape[0]
    src, dst = ei[0], ei[1]

    def conv(h, W, b):
        z = h @ W
        msg = z[src] * w[:, None]
        agg = np.zeros((N, z.shape[1]), np.float32)
        np.add.at(agg, dst, msg)
        return agg + b

    h1 = np.maximum(conv(x, W1, b1), 0.0)
    h2 = conv(h1, W2, b2)
    q = np.concatenate([h2[qe[:, 0]], h2[qe[:, 1]]], axis=1)
    logits = q @ Wl + bl
    m = logits.max(axis=1, keepdims=True)
    e = np.exp(logits - m)
    return logits - m - np.log(e.sum(axis=1, keepdims=True))

# ----------------------------------------------------------------------------
# Entry point: full inputs in, full output out
# ----------------------------------------------------------------------------

LAST_RESULTS = None


def kernel(**inputs):
    """Takes the FULL (unsharded) inputs of nn_DiGCN_link_prediction and
    returns the full [N_QUERY, 2] float32 log-softmax output.

    Shards nodes/edges/queries across 8 NeuronCores internally, runs one
    SPMD Bass kernel (per-core data, identical graph), and reassembles.
    """
    global LAST_RESULTS
    import os
    from concourse.bass_utils import run_bass_kernel_spmd

    n_cores = 8
    dims, in_maps, meta = plan(inputs, n_cores=n_cores)
    nc = build_nc(dims)
    res = run_bass_kernel_spmd(
        nc, in_maps, core_ids=list(range(n_cores)),
        trace=bool(int(os.environ.get("GNN_TRACE", "0"))),
        stitch_traces=False,
    )
    LAST_RESULTS = res
    return unshard(res.results, meta)
